# revision 47
# baseline (speedup 1.0000x reference)
"""Trainium2 Bass kernel for nn_Attention_65343632441735 (XCA-style channel
attention: 1x1 conv -> depthwise 3x3 -> channel attention -> 1x1 proj).

Sharding: data-parallel over batch (8 images, 1 per NeuronCore). Inside each
core: 1x1 conv on PE; depthwise 3x3 split DVE (chunks 0-1, mul+add in 2x/4x
perf modes) / PE diag-matmul (chunks 2-4); channel norms as sum-of-squares on
ACT; q@k gram on PE; fused (Wproj @ attn) @ v output stage with wide DMAs.
"""

import numpy as np
import ml_dtypes

import concourse.bass as bass
import concourse.tile as tile
from concourse import mybir
from concourse.bass_utils import run_bass_kernel_spmd

F32 = mybir.dt.float32
BF16 = mybir.dt.bfloat16
AL = mybir.AluOpType
ACTF = mybir.ActivationFunctionType

C = 192          # input channels
OC = 576         # 3*C qkv channels
HEADS = 4
CH = 48          # channels per head
W = 128          # image width (one row = one 128-px chunk)
EPS = 1e-12

# oc chunking: 4 full 128-chunks + one 64-chunk
OCW = [128, 128, 128, 128, 64]
PE_CHUNKS = (2, 3, 4)    # chunks with a PE-diag accumulation pass
TAPS = [(di, dj) for di in (-1, 0, 1) for dj in (-1, 0, 1)]
# per-chunk engine for each of the 9 depthwise taps:
#   D = DVE mul+add, A = ACT-scaled-copy + DVE add, P = PE diag matmul
ASSIGN = {
    0: "DDDDDDDDD",
    1: "DDDDDDDDD",
    2: "PPPPPPPPP",
    3: "PPPPPPPPP",
    4: "PPPPPPPPP",
}
# engine for the PSUM->SBUF copy after conv / PE-diag dw, per chunk
CONV_COPY_ENG = {0: "A", 1: "A", 2: "A", 3: "A", 4: "A"}
DW_COPY_ENG = {2: "A", 3: "A", 4: "A"}


def _bf(a):
    return np.ascontiguousarray(a.astype(ml_dtypes.bfloat16))


def prep_weights(w_qkv, w_dw, w_proj, temperature):
    wqkvT = _bf(w_qkv[:, :, 0, 0].T)                       # [192, 576]
    dwv = np.zeros((128, 5, 9), np.float32)                # per-partition taps
    for m in range(5):
        ow = OCW[m]
        for t in range(9):
            di, dj = TAPS[t]
            dwv[:ow, m, t] = w_dw[128 * m:128 * m + ow, 0, di + 1, dj + 1]
    dgm = np.zeros((128, 3, 9, 128), np.float32)           # diag mats, PE chunks
    for ci, m in enumerate(PE_CHUNKS):
        ow = OCW[m]
        for t in range(9):
            di, dj = TAPS[t]
            np.fill_diagonal(dgm[:ow, ci, t, :ow],
                             w_dw[128 * m:128 * m + ow, 0, di + 1, dj + 1])
    eye96 = np.eye(96, dtype=np.float32)
    ones1 = np.ones((1, 96), np.float32)
    # additive mask: 0 on the two 48x48 diagonal blocks, -1e30 off-diagonal
    blkmask = np.full((96, 96), -1e30, np.float32)
    blkmask[0:48, 0:48] = 0.0
    blkmask[48:96, 48:96] = 0.0
    # wproj rows grouped by head-pair: wpjp[c, p, o] = wprojT[96p + c, o]
    wpjp = _bf(w_proj[:, :, 0, 0].T.reshape(2, 96, C).transpose(1, 0, 2))
    # temperature per head-pair block: temps96[r, p] = temperature[2p + r//48]
    t = temperature.reshape(HEADS)
    temps96 = np.zeros((96, 2), np.float32)
    for p in range(2):
        temps96[0:48, p] = t[2 * p]
        temps96[48:96, p] = t[2 * p + 1]
    return {
        "wqkvT": wqkvT, "dwv": dwv, "dgm": _bf(dgm), "eye96": eye96,
        "ones1": ones1, "wpjp": wpjp, "temps96": temps96, "blkmask": blkmask,
    }


def build_nc(H=128, legalize=True):
    """One-core program; every core runs it on its own image."""
    assert H % 16 == 0
    NS = H // 16            # slabs of 16 rows
    HW = H * W

    nc = bass.Bass("TRN2")
    x_d = nc.dram_tensor("x", (C, H, W), F32, kind="ExternalInput")
    f_d = nc.dram_tensor("f", (C, H, W), F32, kind="ExternalInput")
    wqkvT_d = nc.dram_tensor("wqkvT", (C, OC), BF16, kind="ExternalInput")
    wpjp_d = nc.dram_tensor("wpjp", (96, 2, C), BF16, kind="ExternalInput")
    dwv_d = nc.dram_tensor("dwv", (128, 5, 9), F32, kind="ExternalInput")
    dgm_d = nc.dram_tensor("dgm", (128, 3, 9, 128), BF16, kind="ExternalInput")
    eye_d = nc.dram_tensor("eye96", (96, 96), F32, kind="ExternalInput")
    ones_d = nc.dram_tensor("ones1", (1, 96), F32, kind="ExternalInput")
    msk_d = nc.dram_tensor("blkmask", (96, 96), F32, kind="ExternalInput")
    tmp_d = nc.dram_tensor("temps96", (96, 2), F32, kind="ExternalInput")
    out_d = nc.dram_tensor("out", (C, H, W), F32, kind="ExternalOutput")

    with tile.TileContext(nc) as tc:
        _body(nc, tc, H, NS, HW, x_d, f_d, wqkvT_d, wpjp_d, dwv_d, dgm_d,
              eye_d, ones_d, msk_d, tmp_d, out_d)
    nc.finalize()
    if legalize:
        legalize_waits(nc)
    return nc


def _row_pairs(ro, nrows):
    """Row-tile groups of up to 8 rows (two 4-row px-tiles per PSUM tile)."""
    out = []
    r = ro
    while r < ro + nrows:
        out.append((r, min(8, ro + nrows - r)))
        r += 8
    return out


def _body(nc, tc, H, NS, HW, x_d, f_d, wqkvT_d, wpjp_d, dwv_d, dgm_d,
          eye_d, ones_d, msk_d, tmp_d, out_d):
    import contextlib
    ctx = contextlib.ExitStack()
    with ctx:
        const = ctx.enter_context(tc.tile_pool(name="const", bufs=1))
        keep = ctx.enter_context(tc.tile_pool(name="keep", bufs=1))
        tail_p = ctx.enter_context(tc.tile_pool(name="tail", bufs=1))

        # ---- constants ----
        wq1 = const.tile([128, OC], BF16)
        wq2 = const.tile([64, OC], BF16)
        nc.sync.dma_start(wq1[:], wqkvT_d[0:128, :])
        nc.sync.dma_start(wq2[:], wqkvT_d[128:C, :])
        wpj = const.tile([96, 2, C], BF16)
        nc.sync.dma_start(wpj[:], wpjp_d[:])
        dwv = const.tile([128, 5, 9], F32)
        nc.sync.dma_start(dwv[:], dwv_d[:])
        dgm = const.tile([128, 3, 9, 128], BF16)
        nc.sync.dma_start(dgm[:], dgm_d[:])
        eye = const.tile([96, 96], F32)
        nc.sync.dma_start(eye[:], eye_d[:])
        ones1 = const.tile([1, 96], F32)
        nc.sync.dma_start(ones1[:], ones_d[:])
        msk = const.tile([96, 96], F32)
        nc.sync.dma_start(msk[:], msk_d[:])
        tmps = const.tile([96, 2], F32)
        nc.sync.dma_start(tmps[:], tmp_d[:])

        vfA = keep.tile([128, H, W], BF16)  # v channels 0..127 (oc 384..511)
        vfB = keep.tile([64, H, W], BF16)   # v channels 128..191 (oc 512..575)
        ssq = keep.tile([128, 3, NS], F32)  # per-(chunk,slab) sum-of-squares
        McA = tail_p.tile([128, C], BF16)  # (W_proj @ attn).T rows = v ch 0..127
        McB = tail_p.tile([64, C], BF16)   # v ch 128..191

        with tc.tile_pool(name="psg", bufs=1, space="PSUM") as psg_p:
            # Gb[:, p, :] = q_pair @ k_pair.T accumulated over all px
            Gb = psg_p.tile([96, 2, 96], F32, tag="G", name="Gb")
            Gp = [Gb[:, p, :] for p in range(2)]

            # ============= stage A: conv + depthwise + gram =============
            with tc.tile_pool(name="xin", bufs=2) as xin_p, \
                 tc.tile_pool(name="xf", bufs=1) as xf_p, \
                 tc.tile_pool(name="pre", bufs=2) as pre_p, \
                 tc.tile_pool(name="scr", bufs=1) as scr_p, \
                 tc.tile_pool(name="scr2", bufs=2) as scr2_p, \
                 tc.tile_pool(name="sq", bufs=1) as sq_p, \
                 tc.tile_pool(name="qkdw", bufs=1) as qkdw_p, \
                 tc.tile_pool(name="qkT", bufs=2) as qkT_p, \
                 tc.tile_pool(name="psA", bufs=4, space="PSUM") as psA_p, \
                 tc.tile_pool(name="psD", bufs=3, space="PSUM") as psD_p:
                for s in range(NS):
                    _slab(nc, s, NS, xin_p, xf_p, pre_p, scr_p, scr2_p, sq_p, qkdw_p,
                          qkT_p, psA_p, psD_p, x_d, f_d, wq1, wq2, dwv, dgm,
                          vfA, vfB, ssq, Gp)

            # ================= attention tail =================
            with tc.tile_pool(name="pst", bufs=1, space="PSUM") as pst_p:
                _tail(nc, NS, tail_p, pst_p, ssq, Gp, eye, ones1, msk, tmps,
                      wpj, McA, McB)

        # ========== stage C: out[o, px] = McA.T @ vfA + McB.T @ vfB ==========
        with tc.tile_pool(name="osb", bufs=2) as osb_p, \
             tc.tile_pool(name="psC", bufs=2, space="PSUM") as psC_p:
            NG = H // 16
            for g in range(NG):
                for mc, (o0, ow) in enumerate(((0, 128), (128, 64))):
                    acc = psC_p.tile([128, 16, W], F32, tag="psC")
                    for ti in range(4):
                        r = 16 * g + 4 * ti
                        nc.tensor.matmul(
                            acc[0:ow, 4 * ti:4 * ti + 4, :],
                            McA[:, o0:o0 + ow], vfA[:, r:r + 4, :],
                            start=True, stop=False, skip_group_check=True)
                        nc.tensor.matmul(
                            acc[0:ow, 4 * ti:4 * ti + 4, :],
                            McB[:, o0:o0 + ow], vfB[:, r:r + 4, :],
                            start=False, stop=True, skip_group_check=True)
                    osb = osb_p.tile([128, 16, W], F32, tag=f"osb{mc}")
                    if mc == 0:
                        nc.scalar.copy(osb[0:ow, :, :], acc[0:ow, :, :])
                    else:
                        nc.vector.tensor_copy(osb[0:ow, :, :], acc[0:ow, :, :])
                    nc.sync.dma_start(out_d[o0:o0 + ow, 16 * g:16 * g + 16, :],
                                      osb[0:ow, :, :])


def _slab(nc, s, NS, xin_p, xf_p, pre_p, scr_p, scr2_p, sq_p, qkdw_p, qkT_p, psA_p,
          psD_p, x_d, f_d, wq1, wq2, dwv, dgm, vfA, vfB, ssq, Gp):
    r0 = 16 * s - 1
    rs, re = max(r0, 0), min(16 * s + 17, 16 * NS)
    nrows = re - rs
    ro = rs - r0  # offset of first loaded row inside 18-row window

    # NOTE: SWDGE accum-DMA corrupts first/last row of the transfer on HW,
    # so load x and f separately (cast DMAs) and add on DVE.
    xin1 = xin_p.tile([128, 18, W], BF16, tag="xin1")
    xin2 = xin_p.tile([64, 18, W], BF16, tag="xin2")
    xf1 = xf_p.tile([128, 18, W], BF16, tag="xf1")
    xf2 = xf_p.tile([64, 18, W], BF16, tag="xf2")
    nc.gpsimd.dma_start(xin1[:, ro:ro + nrows, :], x_d[0:128, rs:re, :])
    nc.gpsimd.dma_start(xf1[:, ro:ro + nrows, :], f_d[0:128, rs:re, :])
    nc.vector.tensor_add(xin1[:, ro:ro + nrows, :],
                         xin1[:, ro:ro + nrows, :],
                         xf1[:, ro:ro + nrows, :])
    nc.gpsimd.dma_start(xin2[:, ro:ro + nrows, :], x_d[128:C, rs:re, :])
    nc.gpsimd.dma_start(xf2[:, ro:ro + nrows, :], f_d[128:C, rs:re, :])
    nc.vector.tensor_add(xin2[:, ro:ro + nrows, :],
                         xin2[:, ro:ro + nrows, :],
                         xf2[:, ro:ro + nrows, :])

    pre = pre_p.tile([128, 5, 18, 130], BF16, tag="pre")
    if s <= 1:
        nc.vector.memset(pre[:, :, :, 0:1], 0.0)
        nc.vector.memset(pre[:, :, :, 129:130], 0.0)
    if s == 0:
        nc.vector.memset(pre[:, :, 0, :], 0.0)
    if s == NS - 1:
        nc.vector.memset(pre[:, :, 17, :], 0.0)

    # 1x1 conv: qkv_pre[oc, px] = wqkvT.T @ x_in, 8-row groups per PSUM tile
    for m in range(5):
        ow = OCW[m]
        for (rt, rw) in _row_pairs(ro, nrows):
            for half in range(2):
                hw_ = min(4, rw - 4 * half)
                if hw_ <= 0:
                    break
                acc = psA_p.tile([128, 4, W], F32, tag="psA")
                r0_ = rt + 4 * half
                nc.tensor.matmul(
                    acc[0:ow, 0:hw_, :],
                    wq1[:, 128 * m:128 * m + ow],
                    xin1[:, r0_:r0_ + hw_, :],
                    start=True, stop=False)
                nc.tensor.matmul(
                    acc[0:ow, 0:hw_, :],
                    wq2[:, 128 * m:128 * m + ow],
                    xin2[:, r0_:r0_ + hw_, :],
                    start=False, stop=True)
                nc.scalar.copy(pre[0:ow, m, r0_:r0_ + hw_, 1:1 + W],
                               acc[0:ow, 0:hw_, :])

    def pre_view(m, di, dj, ow, rbase=1, nr=16):
        return pre[0:ow, m, rbase + di:rbase + di + nr, 1 + dj:1 + dj + W]

    # depthwise 3x3, engine split per ASSIGN:
    #   P taps accumulate on PE (diag matmuls in PSUM, copied out below)
    #   D taps: DVE scaled-mul (4x mode) + DVE add (2x mode)
    #   A taps: ACT scaled-copy + DVE add
    qkdw = qkdw_p.tile([128, 3, 16, W], BF16, tag="qkdw")
    for m in range(3):
        ow = OCW[m]
        dst = qkdw[0:ow, m, :, :]
        first = True
        for t, (di, dj) in enumerate(TAPS):
            kind = ASSIGN[m][t]
            if kind == "P":
                first = False  # PE partial already copied into dst
                continue
            src = pre_view(m, di, dj, ow)
            sc = dwv[0:ow, m, t:t + 1]
            if kind == "D" and first:
                nc.vector.tensor_scalar_mul(dst, src, sc)
            elif kind == "D":
                scr = scr_p.tile([128, 16, W], BF16, tag="scr")
                nc.vector.tensor_scalar_mul(scr[0:ow, :, :], src, sc)
                nc.vector.tensor_add(dst, dst, scr[0:ow, :, :])
            else:  # ACT-assisted
                sc2 = scr2_p.tile([128, 16, W], BF16, tag="scr2")
                nc.scalar.activation(out=sc2[0:ow, :, :], in_=src,
                                     func=ACTF.Copy, scale=sc)
                nc.vector.tensor_add(dst, dst, sc2[0:ow, :, :])
            first = False

    for ci, m in enumerate(PE_CHUNKS):
        ow = OCW[m]
        pe_taps = [t for t in range(9) if ASSIGN[m][t] == "P"]
        for pt in range(4):
            acc = psD_p.tile([128, 4, W], F32, tag="psD")
            for i, t in enumerate(pe_taps):
                di, dj = TAPS[t]
                nc.tensor.matmul(
                    acc[0:ow, :, :],
                    dgm[0:ow, ci, t, 0:ow],
                    pre_view(m, di, dj, ow, rbase=1 + 4 * pt, nr=4),
                    start=(i == 0), stop=(i == len(pe_taps) - 1))
            vr = 16 * s + 4 * pt
            if m == 2:
                dst = qkdw[:, 2, 4 * pt:4 * pt + 4, :]
            elif m == 3:
                dst = vfA[:, vr:vr + 4, :]
            else:
                dst = vfB[:, vr:vr + 4, :]
            if DW_COPY_ENG[m] == "A":
                nc.scalar.copy(dst, acc[0:ow, :, :])
            else:
                nc.gpsimd.tensor_copy(dst, acc[0:ow, :, :])

    # sum-of-squares per channel for q,k norms (ACT, accumulated per slab)
    for m in range(3):
        sqs = scr2_p.tile([128, 16, W], BF16, tag="scr2")
        nc.scalar.activation(out=sqs[:], in_=qkdw[:, m, :, :],
                             func=ACTF.Square,
                             accum_out=ssq[:, m, s:s + 1])

    # transpose q,k slab -> [px, ch] layout (half-slab granularity so the
    # gram can start before the last dw chunk is fully transposed)
    qkT = qkT_p.tile([128, 16, 384], BF16, tag="qkT")
    for h in range(2):
        for m in range(3):
            nc.sync.dma_start_transpose(
                qkT[:, 8 * h:8 * h + 8, 128 * m:128 * (m + 1)],
                qkdw[:, m, 8 * h:8 * h + 8, :])
        for pc in range(8 * h, 8 * h + 8):
            st = (s == 0 and pc == 0)
            sp = (s == NS - 1 and pc == 15)
            for p in range(2):
                qs = qkT[:, pc, 96 * p:96 * p + 96]
                ks = qkT[:, pc, 192 + 96 * p:192 + 96 * p + 96]
                nc.tensor.matmul(Gp[p], qs, ks, start=st, stop=sp,
                                 skip_group_check=True)


def _tail(nc, NS, tail_p, pst_p, ssq, Gp, eye, ones1, msk, tmps, wpj,
          McA, McB):
    # reduce per-slab partials -> per-channel sumsq in chunk layout
    ss3 = tail_p.tile([128, 3], F32)
    for m in range(3):
        nc.vector.tensor_reduce(ss3[:, m:m + 1], ssq[:, m, :],
                                axis=mybir.AxisListType.X, op=AL.add)
    # rearrange chunk layout -> pair layout [96, (q0, q1, k0, k1)]
    nsq = tail_p.tile([96, 4], F32)
    nc.vector.tensor_copy(nsq[:, 0:1], ss3[0:96, 0:1])
    nc.sync.dma_start(nsq[0:32, 1:2], ss3[96:128, 0:1])
    nc.gpsimd.dma_start(nsq[32:96, 1:2], ss3[0:64, 1:2])
    nc.sync.dma_start(nsq[0:64, 2:3], ss3[64:128, 1:2])
    nc.gpsimd.dma_start(nsq[64:96, 2:3], ss3[0:32, 2:3])
    nc.sync.dma_start(nsq[0:96, 3:4], ss3[32:128, 2:3])
    nrm = tail_p.tile([96, 4], F32)
    nc.scalar.activation(nrm[:], nsq[:], ACTF.Sqrt)
    nc.vector.tensor_scalar_max(nrm[:], nrm[:], EPS)
    rn = tail_p.tile([96, 4], F32)
    nc.vector.reciprocal(rn[:], nrm[:])

    abps = []
    for p in range(2):
        # k-norm reciprocals along the free dim: [96,1] -T-> [1,96] -> bcast
        rT_ps = pst_p.tile([1, 96], F32, tag=f"rT{p}", name=f"rT{p}")
        nc.tensor.transpose(rT_ps[:], rn[:, 2 + p:3 + p], eye[:])
        rT = tail_p.tile([1, 96], F32, tag=f"rTs{p}", name=f"rTs{p}")
        nc.scalar.copy(rT[:], rT_ps[:])
        bc_ps = pst_p.tile([96, 96], F32, tag=f"bc{p}", name=f"bc{p}")
        nc.tensor.matmul(bc_ps[:], ones1[:], rT[:], start=True, stop=True,
                         skip_group_check=True)
        at = tail_p.tile([96, 96], F32, tag=f"at{p}", name=f"at{p}")
        nc.vector.tensor_scalar_mul(at[:], Gp[p], rn[:, p:p + 1])
        nc.vector.tensor_mul(at[:], at[:], bc_ps[:])
        nc.vector.tensor_add(at[:], at[:], msk[:])
        mx = tail_p.tile([96, 1], F32, tag=f"mx{p}", name=f"mx{p}")
        nc.vector.tensor_reduce(mx[:], at[:], axis=mybir.AxisListType.X,
                                op=AL.max)
        mb = tail_p.tile([96, 1], F32, tag=f"mb{p}", name=f"mb{p}")
        nc.vector.tensor_scalar(out=mb[:], in0=mx[:],
                                scalar1=tmps[:, p:p + 1], scalar2=-1.0,
                                op0=AL.mult, op1=AL.mult)
        ae = tail_p.tile([96, 96], F32, tag=f"ae{p}", name=f"ae{p}")
        se = tail_p.tile([96, 1], F32, tag=f"se{p}", name=f"se{p}")
        nc.scalar.activation(out=ae[:], in_=at[:], func=ACTF.Exp,
                             bias=mb[:], scale=tmps[:, p:p + 1],
                             accum_out=se[:])
        rs_ = tail_p.tile([96, 1], F32, tag=f"rs{p}", name=f"rs{p}")
        nc.vector.reciprocal(rs_[:], se[:])
        abp = tail_p.tile([96, 96], BF16, tag=f"abp{p}", name=f"abp{p}")
        nc.vector.tensor_scalar_mul(abp[:], ae[:], rs_[:])
        abps.append(abp)

    # M_p[d, o] = sum_c abp_p[c, d] * wpjp[c, p, o]; assemble into v-chunk
    # layout: McA rows = v ch 0..127, McB rows = v ch 128..191
    mh = []
    for p in range(2):
        mh_ps = pst_p.tile([96, C], F32, tag=f"mh{p}", name=f"mh{p}")
        nc.tensor.matmul(mh_ps[:], abps[p][:], wpj[:, p, :], start=True,
                         stop=True, skip_group_check=True)
        mh.append(mh_ps)
    nc.scalar.copy(McA[0:96, :], mh[0][:])
    # pair1 rows 0..31 -> McA partitions 96..127; rows 32..95 -> McB 0..63
    mh1 = tail_p.tile([96, C], BF16)
    nc.vector.tensor_copy(mh1[:], mh[1][:])
    nc.sync.dma_start(McA[96:128, :], mh1[0:32, :])
    nc.gpsimd.dma_start(McB[0:64, :], mh1[32:96, :])


def legalize_waits(nc):
    """This walrus build encodes at most ONE sync-wait per instruction (none on
    Drain): hoist extras onto injected single-wait NoOps."""
    n_fix = 0
    for fn in nc.m.functions:
        for bb in fn.blocks:
            insts = list(bb.instructions)
            new_insts = []
            changed = False
            for ins in insts:
                si = ins.sync_info
                waits = list(si.on_wait) if si is not None else []
                keep = 0 if type(ins).__name__ == "InstDrain" else 1
                if len(waits) > keep:
                    n_hoist = len(waits) - keep
                    hoisted, kept = waits[:n_hoist], waits[n_hoist:]
                    for j, w in enumerate(hoisted):
                        new_insts.append(mybir.InstNoOp(
                            name=f"{ins.name}_hw{j}", engine=ins.engine,
                            sync_info=mybir.SyncInfo(on_wait=[w], on_update=[]),
                            bass_nofuse=True))
                        n_fix += 1
                    ins.sync_info = mybir.SyncInfo(
                        on_wait=kept, on_update=list(si.on_update) if si else [])
                    changed = True
                new_insts.append(ins)
            if changed:
                try:
                    bb.instructions = new_insts
                except Exception:
                    bb.instructions.clear()
                    bb.instructions.extend(new_insts)
    return n_fix


_NC_CACHE = {}


def _get_nc(H):
    if H not in _NC_CACHE:
        _NC_CACHE[H] = build_nc(H)
    return _NC_CACHE[H]


def kernel(x, f, w_qkv, w_dw, w_proj, temperature, _H=None, _trace=False):
    x = np.asarray(x, np.float32)
    f = np.asarray(f, np.float32)
    b = x.shape[0]
    H = x.shape[2] if _H is None else _H
    wts = prep_weights(np.asarray(w_qkv, np.float32),
                       np.asarray(w_dw, np.float32),
                       np.asarray(w_proj, np.float32),
                       np.asarray(temperature, np.float32))
    nc = _get_nc(H)
    in_maps = []
    for i in range(b):
        m = {"x": np.ascontiguousarray(x[i]),
             "f": np.ascontiguousarray(f[i])}
        m.update(wts)
        in_maps.append(m)
    res = run_bass_kernel_spmd(nc, in_maps, core_ids=list(range(b)),
                               trace=_trace)
    out = np.stack([res.results[i]["out"] for i in range(b)], axis=0)
    kernel.last_results = res
    return out


# revision 50
# speedup vs baseline: 1.2323x; 1.2323x over previous
"""Trainium2 Bass kernel for nn_Attention_65343632441735 (XCA-style channel
attention: 1x1 conv -> depthwise 3x3 -> channel attention -> 1x1 proj).

Sharding: data-parallel over batch (8 images, 1 per NeuronCore). Inside each
core: 1x1 conv on PE; depthwise 3x3 split DVE (chunks 0-1, mul+add in 2x/4x
perf modes) / PE diag-matmul (chunks 2-4); channel norms as sum-of-squares on
ACT; q@k gram on PE; fused (Wproj @ attn) @ v output stage with wide DMAs.
"""

import numpy as np
import ml_dtypes

import concourse.bass as bass
import concourse.tile as tile
from concourse import mybir
from concourse.bass_utils import run_bass_kernel_spmd

F32 = mybir.dt.float32
BF16 = mybir.dt.bfloat16
AL = mybir.AluOpType
ACTF = mybir.ActivationFunctionType

C = 192          # input channels
OC = 576         # 3*C qkv channels
HEADS = 4
CH = 48          # channels per head
W = 128          # image width (one row = one 128-px chunk)
EPS = 1e-12

# oc chunking: 4 full 128-chunks + one 64-chunk
OCW = [128, 128, 128, 128, 64]
PE_CHUNKS = (2, 3, 4)    # chunks with a PE-diag accumulation pass
TAPS = [(di, dj) for di in (-1, 0, 1) for dj in (-1, 0, 1)]
# per-chunk engine for each of the 9 depthwise taps:
#   D = DVE mul+add, A = ACT-scaled-copy + DVE add, P = PE diag matmul
ASSIGN = {
    0: "DDDDDDDDD",
    1: "DDDDDDDDD",
    2: "PPPPPPPPP",
    3: "PPPPPPPPP",
    4: "PPPPPPPPP",
}
# engine for the PSUM->SBUF copy after conv / PE-diag dw, per chunk
CONV_COPY_ENG = {0: "A", 1: "A", 2: "A", 3: "A", 4: "A"}
DW_COPY_ENG = {2: "A", 3: "A", 4: "A"}


def _bf(a):
    return np.ascontiguousarray(a.astype(ml_dtypes.bfloat16))


def prep_weights(w_qkv, w_dw, w_proj, temperature):
    wqkvT = _bf(w_qkv[:, :, 0, 0].T)                       # [192, 576]
    dwv = np.zeros((128, 5, 9), np.float32)                # per-partition taps
    for m in range(5):
        ow = OCW[m]
        for t in range(9):
            di, dj = TAPS[t]
            dwv[:ow, m, t] = w_dw[128 * m:128 * m + ow, 0, di + 1, dj + 1]
    dgm = np.zeros((128, 3, 9, 128), np.float32)           # diag mats, PE chunks
    for ci, m in enumerate(PE_CHUNKS):
        ow = OCW[m]
        for t in range(9):
            di, dj = TAPS[t]
            np.fill_diagonal(dgm[:ow, ci, t, :ow],
                             w_dw[128 * m:128 * m + ow, 0, di + 1, dj + 1])
    eye96 = np.eye(96, dtype=np.float32)
    ones1 = np.ones((1, 96), np.float32)
    # additive mask: 0 on the two 48x48 diagonal blocks, -1e30 off-diagonal
    blkmask = np.full((96, 96), -1e30, np.float32)
    blkmask[0:48, 0:48] = 0.0
    blkmask[48:96, 48:96] = 0.0
    # wproj rows grouped by head-pair: wpjp[c, p, o] = wprojT[96p + c, o]
    wpjp = _bf(w_proj[:, :, 0, 0].T.reshape(2, 96, C).transpose(1, 0, 2))
    # temperature per head-pair block: temps96[r, p] = temperature[2p + r//48]
    t = temperature.reshape(HEADS)
    temps96 = np.zeros((96, 2), np.float32)
    for p in range(2):
        temps96[0:48, p] = t[2 * p]
        temps96[48:96, p] = t[2 * p + 1]
    return {
        "wqkvT": wqkvT, "dwv": dwv, "dgm": _bf(dgm), "eye96": eye96,
        "ones1": ones1, "wpjp": wpjp, "temps96": temps96, "blkmask": blkmask,
    }


def build_nc(H=128, legalize=True):
    """One-core program; every core runs it on its own image."""
    assert H % 16 == 0
    NS = H // 16            # slabs of 16 rows
    HW = H * W

    nc = bass.Bass("TRN2")
    x_d = nc.dram_tensor("x", (C, H, W), F32, kind="ExternalInput")
    f_d = nc.dram_tensor("f", (C, H, W), F32, kind="ExternalInput")
    wqkvT_d = nc.dram_tensor("wqkvT", (C, OC), BF16, kind="ExternalInput")
    wpjp_d = nc.dram_tensor("wpjp", (96, 2, C), BF16, kind="ExternalInput")
    dwv_d = nc.dram_tensor("dwv", (128, 5, 9), F32, kind="ExternalInput")
    dgm_d = nc.dram_tensor("dgm", (128, 3, 9, 128), BF16, kind="ExternalInput")
    eye_d = nc.dram_tensor("eye96", (96, 96), F32, kind="ExternalInput")
    ones_d = nc.dram_tensor("ones1", (1, 96), F32, kind="ExternalInput")
    msk_d = nc.dram_tensor("blkmask", (96, 96), F32, kind="ExternalInput")
    tmp_d = nc.dram_tensor("temps96", (96, 2), F32, kind="ExternalInput")
    out_d = nc.dram_tensor("out", (C, H, W), F32, kind="ExternalOutput")

    with tile.TileContext(nc) as tc:
        _body(nc, tc, H, NS, HW, x_d, f_d, wqkvT_d, wpjp_d, dwv_d, dgm_d,
              eye_d, ones_d, msk_d, tmp_d, out_d)
    nc.finalize()
    if legalize:
        legalize_waits(nc)
    return nc


def _row_pairs(ro, nrows):
    """Row-tile groups of up to 8 rows (two 4-row px-tiles per PSUM tile)."""
    out = []
    r = ro
    while r < ro + nrows:
        out.append((r, min(8, ro + nrows - r)))
        r += 8
    return out


def _body(nc, tc, H, NS, HW, x_d, f_d, wqkvT_d, wpjp_d, dwv_d, dgm_d,
          eye_d, ones_d, msk_d, tmp_d, out_d):
    import contextlib
    ctx = contextlib.ExitStack()
    with ctx:
        const = ctx.enter_context(tc.tile_pool(name="const", bufs=1))
        keep = ctx.enter_context(tc.tile_pool(name="keep", bufs=1))
        tail_p = ctx.enter_context(tc.tile_pool(name="tail", bufs=1))

        # ---- constants ----
        wq1 = const.tile([128, OC], BF16)
        wq2 = const.tile([64, OC], BF16)
        nc.sync.dma_start(wq1[:], wqkvT_d[0:128, :])
        nc.sync.dma_start(wq2[:], wqkvT_d[128:C, :])
        wpj = const.tile([96, 2, C], BF16)
        nc.sync.dma_start(wpj[:], wpjp_d[:])
        dwv = const.tile([128, 5, 9], F32)
        nc.sync.dma_start(dwv[:], dwv_d[:])
        dgm = const.tile([128, 3, 9, 128], BF16)
        nc.sync.dma_start(dgm[:], dgm_d[:])
        eye = const.tile([96, 96], F32)
        nc.sync.dma_start(eye[:], eye_d[:])
        ones1 = const.tile([1, 96], F32)
        nc.sync.dma_start(ones1[:], ones_d[:])
        msk = const.tile([96, 96], F32)
        nc.sync.dma_start(msk[:], msk_d[:])
        tmps = const.tile([96, 2], F32)
        nc.sync.dma_start(tmps[:], tmp_d[:])

        vfA = keep.tile([128, H, W], BF16)  # v channels 0..127 (oc 384..511)
        vfB = keep.tile([64, H, W], BF16)   # v channels 128..191 (oc 512..575)
        ssq = keep.tile([128, 3, NS], F32)  # per-(chunk,slab) sum-of-squares
        McA = tail_p.tile([128, C], BF16)  # (W_proj @ attn).T rows = v ch 0..127
        McB = tail_p.tile([64, C], BF16)   # v ch 128..191

        with tc.tile_pool(name="psg", bufs=1, space="PSUM") as psg_p:
            # Gb[:, p, :] = q_pair @ k_pair.T accumulated over all px
            Gb = psg_p.tile([96, 2, 96], F32, tag="G", name="Gb")
            Gp = [Gb[:, p, :] for p in range(2)]

            # ============= stage A: conv + depthwise + gram =============
            with tc.tile_pool(name="xin", bufs=2) as xin_p, \
                 tc.tile_pool(name="xf", bufs=1) as xf_p, \
                 tc.tile_pool(name="pre", bufs=2) as pre_p, \
                 tc.tile_pool(name="scr", bufs=1) as scr_p, \
                 tc.tile_pool(name="scr2", bufs=2) as scr2_p, \
                 tc.tile_pool(name="qkdw", bufs=1) as qkdw_p, \
                 tc.tile_pool(name="qkT", bufs=2) as qkT_p, \
                 tc.tile_pool(name="psA", bufs=4, space="PSUM") as psA_p, \
                 tc.tile_pool(name="psD", bufs=3, space="PSUM") as psD_p:
                for s in range(NS):
                    _slab(nc, s, NS, xin_p, xf_p, pre_p, scr_p, scr2_p,
                          qkdw_p, qkT_p, psA_p, psD_p, x_d, f_d, wq1, wq2,
                          dwv, dgm, vfA, vfB, ssq, Gp)

            # ================= attention tail =================
            with tc.tile_pool(name="pst", bufs=1, space="PSUM") as pst_p:
                _tail(nc, NS, tail_p, pst_p, ssq, Gp, eye, ones1, msk, tmps,
                      wpj, McA, McB)

        # ========== stage C: out[o, px] = McA.T @ vfA + McB.T @ vfB ==========
        with tc.tile_pool(name="osb", bufs=2) as osb_p, \
             tc.tile_pool(name="psC", bufs=2, space="PSUM") as psC_p:
            NG = H // 16
            for g in range(NG):
                for mc, (o0, ow) in enumerate(((0, 128), (128, 64))):
                    acc = psC_p.tile([128, 16, W], F32, tag="psC")
                    for ti in range(4):
                        r = 16 * g + 4 * ti
                        nc.tensor.matmul(
                            acc[0:ow, 4 * ti:4 * ti + 4, :],
                            McA[:, o0:o0 + ow], vfA[:, r:r + 4, :],
                            start=True, stop=False, skip_group_check=True)
                        nc.tensor.matmul(
                            acc[0:ow, 4 * ti:4 * ti + 4, :],
                            McB[:, o0:o0 + ow], vfB[:, r:r + 4, :],
                            start=False, stop=True, skip_group_check=True)
                    osb = osb_p.tile([128, 16, W], F32, tag=f"osb{mc}")
                    if mc == 0:
                        nc.scalar.copy(osb[0:ow, :, :], acc[0:ow, :, :])
                    else:
                        nc.vector.tensor_copy(osb[0:ow, :, :], acc[0:ow, :, :])
                    nc.sync.dma_start(out_d[o0:o0 + ow, 16 * g:16 * g + 16, :],
                                      osb[0:ow, :, :])


def _slab(nc, s, NS, xin_p, xf_p, pre_p, scr_p, scr2_p, qkdw_p, qkT_p, psA_p,
          psD_p, x_d, f_d, wq1, wq2, dwv, dgm, vfA, vfB, ssq, Gp):
    r0 = 16 * s - 1
    rs, re = max(r0, 0), min(16 * s + 17, 16 * NS)
    nrows = re - rs
    ro = rs - r0  # offset of first loaded row inside 18-row window

    # NOTE: SWDGE accum-DMA corrupts first/last row of the transfer on HW,
    # so load x and f separately (cast DMAs) and add on DVE.
    xin1 = xin_p.tile([128, 18, W], BF16, tag="xin1")
    xin2 = xin_p.tile([64, 18, W], BF16, tag="xin2")
    xf1 = xf_p.tile([128, 18, W], BF16, tag="xf1")
    xf2 = xf_p.tile([64, 18, W], BF16, tag="xf2")
    nc.gpsimd.dma_start(xin1[:, ro:ro + nrows, :], x_d[0:128, rs:re, :])
    nc.gpsimd.dma_start(xf1[:, ro:ro + nrows, :], f_d[0:128, rs:re, :])
    nc.vector.tensor_add(xin1[:, ro:ro + nrows, :],
                         xin1[:, ro:ro + nrows, :],
                         xf1[:, ro:ro + nrows, :])
    nc.gpsimd.dma_start(xin2[:, ro:ro + nrows, :], x_d[128:C, rs:re, :])
    nc.gpsimd.dma_start(xf2[:, ro:ro + nrows, :], f_d[128:C, rs:re, :])
    nc.vector.tensor_add(xin2[:, ro:ro + nrows, :],
                         xin2[:, ro:ro + nrows, :],
                         xf2[:, ro:ro + nrows, :])

    pre = pre_p.tile([128, 5, 18, 130], BF16, tag="pre")
    if s <= 1:
        nc.vector.memset(pre[:, :, :, 0:1], 0.0)
        nc.vector.memset(pre[:, :, :, 129:130], 0.0)
    if s == 0:
        nc.vector.memset(pre[:, :, 0, :], 0.0)
    if s == NS - 1:
        nc.vector.memset(pre[:, :, 17, :], 0.0)

    # 1x1 conv: qkv_pre[oc, px] = wqkvT.T @ x_in, 8-row groups per PSUM tile
    for m in range(5):
        ow = OCW[m]
        for (rt, rw) in _row_pairs(ro, nrows):
            for half in range(2):
                hw_ = min(4, rw - 4 * half)
                if hw_ <= 0:
                    break
                acc = psA_p.tile([128, 4, W], F32, tag="psA")
                r0_ = rt + 4 * half
                nc.tensor.matmul(
                    acc[0:ow, 0:hw_, :],
                    wq1[:, 128 * m:128 * m + ow],
                    xin1[:, r0_:r0_ + hw_, :],
                    start=True, stop=False)
                nc.tensor.matmul(
                    acc[0:ow, 0:hw_, :],
                    wq2[:, 128 * m:128 * m + ow],
                    xin2[:, r0_:r0_ + hw_, :],
                    start=False, stop=True)
                nc.scalar.copy(pre[0:ow, m, r0_:r0_ + hw_, 1:1 + W],
                               acc[0:ow, 0:hw_, :])

    def pre_view(m, di, dj, ow, rbase=1, nr=16):
        return pre[0:ow, m, rbase + di:rbase + di + nr, 1 + dj:1 + dj + W]

    # depthwise 3x3, engine split per ASSIGN:
    #   P taps accumulate on PE (diag matmuls in PSUM, copied out below)
    #   D taps: DVE scaled-mul (4x mode) + DVE add (2x mode)
    #   A taps: ACT scaled-copy + DVE add
    qkdw = qkdw_p.tile([128, 3, 16, W], BF16, tag="qkdw")
    for m in range(3):
        ow = OCW[m]
        dst = qkdw[0:ow, m, :, :]
        first = True
        for t, (di, dj) in enumerate(TAPS):
            kind = ASSIGN[m][t]
            if kind == "P":
                first = False  # PE partial already copied into dst
                continue
            src = pre_view(m, di, dj, ow)
            sc = dwv[0:ow, m, t:t + 1]
            if kind == "D" and first:
                nc.vector.tensor_scalar_mul(dst, src, sc)
            elif kind == "D":
                scr = scr_p.tile([128, 16, W], BF16, tag="scr")
                nc.vector.tensor_scalar_mul(scr[0:ow, :, :], src, sc)
                nc.vector.tensor_add(dst, dst, scr[0:ow, :, :])
            else:  # ACT-assisted
                sc2 = scr2_p.tile([128, 16, W], BF16, tag="scr2")
                nc.scalar.activation(out=sc2[0:ow, :, :], in_=src,
                                     func=ACTF.Copy, scale=sc)
                nc.vector.tensor_add(dst, dst, sc2[0:ow, :, :])
            first = False

    for ci, m in enumerate(PE_CHUNKS):
        ow = OCW[m]
        pe_taps = [t for t in range(9) if ASSIGN[m][t] == "P"]
        for pt in range(4):
            acc = psD_p.tile([128, 4, W], F32, tag="psD")
            for i, t in enumerate(pe_taps):
                di, dj = TAPS[t]
                nc.tensor.matmul(
                    acc[0:ow, :, :],
                    dgm[0:ow, ci, t, 0:ow],
                    pre_view(m, di, dj, ow, rbase=1 + 4 * pt, nr=4),
                    start=(i == 0), stop=(i == len(pe_taps) - 1))
            vr = 16 * s + 4 * pt
            if m == 2:
                dst = qkdw[:, 2, 4 * pt:4 * pt + 4, :]
            elif m == 3:
                dst = vfA[:, vr:vr + 4, :]
            else:
                dst = vfB[:, vr:vr + 4, :]
            if DW_COPY_ENG[m] == "A":
                nc.scalar.copy(dst, acc[0:ow, :, :])
            else:
                nc.gpsimd.tensor_copy(dst, acc[0:ow, :, :])

    # sum-of-squares per channel for q,k norms (ACT, accumulated per slab)
    for m in range(3):
        sqs = scr2_p.tile([128, 16, W], BF16, tag="scr2")
        nc.scalar.activation(out=sqs[:], in_=qkdw[:, m, :, :],
                             func=ACTF.Square,
                             accum_out=ssq[:, m, s:s + 1])

    # transpose q,k slab -> [px, ch] layout (half-slab granularity so the
    # gram can start before the last dw chunk is fully transposed)
    qkT = qkT_p.tile([128, 16, 384], BF16, tag="qkT")
    for h in range(2):
        for m in range(3):
            nc.sync.dma_start_transpose(
                qkT[:, 8 * h:8 * h + 8, 128 * m:128 * (m + 1)],
                qkdw[:, m, 8 * h:8 * h + 8, :])
        for pc in range(8 * h, 8 * h + 8):
            st = (s == 0 and pc == 0)
            sp = (s == NS - 1 and pc == 15)
            for p in range(2):
                qs = qkT[:, pc, 96 * p:96 * p + 96]
                ks = qkT[:, pc, 192 + 96 * p:192 + 96 * p + 96]
                nc.tensor.matmul(Gp[p], qs, ks, start=st, stop=sp,
                                 skip_group_check=True)


def _tail(nc, NS, tail_p, pst_p, ssq, Gp, eye, ones1, msk, tmps, wpj,
          McA, McB):
    # reduce per-slab partials -> per-channel sumsq in chunk layout
    ss3 = tail_p.tile([128, 3], F32)
    for m in range(3):
        nc.vector.tensor_reduce(ss3[:, m:m + 1], ssq[:, m, :],
                                axis=mybir.AxisListType.X, op=AL.add)
    # rearrange chunk layout -> pair layout [96, (q0, q1, k0, k1)]
    nsq = tail_p.tile([96, 4], F32)
    nc.vector.tensor_copy(nsq[:, 0:1], ss3[0:96, 0:1])
    nc.sync.dma_start(nsq[0:32, 1:2], ss3[96:128, 0:1])
    nc.gpsimd.dma_start(nsq[32:96, 1:2], ss3[0:64, 1:2])
    nc.sync.dma_start(nsq[0:64, 2:3], ss3[64:128, 1:2])
    nc.gpsimd.dma_start(nsq[64:96, 2:3], ss3[0:32, 2:3])
    nc.sync.dma_start(nsq[0:96, 3:4], ss3[32:128, 2:3])
    nrm = tail_p.tile([96, 4], F32)
    nc.scalar.activation(nrm[:], nsq[:], ACTF.Sqrt)
    nc.vector.tensor_scalar_max(nrm[:], nrm[:], EPS)
    rn = tail_p.tile([96, 4], F32)
    nc.vector.reciprocal(rn[:], nrm[:])

    abps = []
    for p in range(2):
        # k-norm reciprocals along the free dim: [96,1] -T-> [1,96] -> bcast
        rT_ps = pst_p.tile([1, 96], F32, tag=f"rT{p}", name=f"rT{p}")
        nc.tensor.transpose(rT_ps[:], rn[:, 2 + p:3 + p], eye[:])
        rT = tail_p.tile([1, 96], F32, tag=f"rTs{p}", name=f"rTs{p}")
        nc.scalar.copy(rT[:], rT_ps[:])
        bc_ps = pst_p.tile([96, 96], F32, tag=f"bc{p}", name=f"bc{p}")
        nc.tensor.matmul(bc_ps[:], ones1[:], rT[:], start=True, stop=True,
                         skip_group_check=True)
        at = tail_p.tile([96, 96], F32, tag=f"at{p}", name=f"at{p}")
        nc.vector.tensor_scalar_mul(at[:], Gp[p], rn[:, p:p + 1])
        nc.vector.tensor_mul(at[:], at[:], bc_ps[:])
        nc.vector.tensor_add(at[:], at[:], msk[:])
        mx = tail_p.tile([96, 1], F32, tag=f"mx{p}", name=f"mx{p}")
        nc.vector.tensor_reduce(mx[:], at[:], axis=mybir.AxisListType.X,
                                op=AL.max)
        mb = tail_p.tile([96, 1], F32, tag=f"mb{p}", name=f"mb{p}")
        nc.vector.tensor_scalar(out=mb[:], in0=mx[:],
                                scalar1=tmps[:, p:p + 1], scalar2=-1.0,
                                op0=AL.mult, op1=AL.mult)
        ae = tail_p.tile([96, 96], F32, tag=f"ae{p}", name=f"ae{p}")
        se = tail_p.tile([96, 1], F32, tag=f"se{p}", name=f"se{p}")
        nc.scalar.activation(out=ae[:], in_=at[:], func=ACTF.Exp,
                             bias=mb[:], scale=tmps[:, p:p + 1],
                             accum_out=se[:])
        rs_ = tail_p.tile([96, 1], F32, tag=f"rs{p}", name=f"rs{p}")
        nc.vector.reciprocal(rs_[:], se[:])
        abp = tail_p.tile([96, 96], BF16, tag=f"abp{p}", name=f"abp{p}")
        nc.vector.tensor_scalar_mul(abp[:], ae[:], rs_[:])
        abps.append(abp)

    # M_p[d, o] = sum_c abp_p[c, d] * wpjp[c, p, o]; assemble into v-chunk
    # layout: McA rows = v ch 0..127, McB rows = v ch 128..191
    mh = []
    for p in range(2):
        mh_ps = pst_p.tile([96, C], F32, tag=f"mh{p}", name=f"mh{p}")
        nc.tensor.matmul(mh_ps[:], abps[p][:], wpj[:, p, :], start=True,
                         stop=True, skip_group_check=True)
        mh.append(mh_ps)
    nc.scalar.copy(McA[0:96, :], mh[0][:])
    # pair1 rows 0..31 -> McA partitions 96..127; rows 32..95 -> McB 0..63
    mh1 = tail_p.tile([96, C], BF16)
    nc.vector.tensor_copy(mh1[:], mh[1][:])
    nc.sync.dma_start(McA[96:128, :], mh1[0:32, :])
    nc.gpsimd.dma_start(McB[0:64, :], mh1[32:96, :])


def legalize_waits(nc):
    """This walrus build encodes at most ONE sync-wait per instruction (none on
    Drain): hoist extras onto injected single-wait NoOps."""
    n_fix = 0
    for fn in nc.m.functions:
        for bb in fn.blocks:
            insts = list(bb.instructions)
            new_insts = []
            changed = False
            for ins in insts:
                si = ins.sync_info
                waits = list(si.on_wait) if si is not None else []
                keep = 0 if type(ins).__name__ == "InstDrain" else 1
                if len(waits) > keep:
                    n_hoist = len(waits) - keep
                    hoisted, kept = waits[:n_hoist], waits[n_hoist:]
                    for j, w in enumerate(hoisted):
                        new_insts.append(mybir.InstNoOp(
                            name=f"{ins.name}_hw{j}", engine=ins.engine,
                            sync_info=mybir.SyncInfo(on_wait=[w], on_update=[]),
                            bass_nofuse=True))
                        n_fix += 1
                    ins.sync_info = mybir.SyncInfo(
                        on_wait=kept, on_update=list(si.on_update) if si else [])
                    changed = True
                new_insts.append(ins)
            if changed:
                try:
                    bb.instructions = new_insts
                except Exception:
                    bb.instructions.clear()
                    bb.instructions.extend(new_insts)
    return n_fix


_NC_CACHE = {}


def _get_nc(H):
    if H not in _NC_CACHE:
        _NC_CACHE[H] = build_nc(H)
    return _NC_CACHE[H]


def kernel(x, f, w_qkv, w_dw, w_proj, temperature, _H=None, _trace=False):
    x = np.asarray(x, np.float32)
    f = np.asarray(f, np.float32)
    b = x.shape[0]
    H = x.shape[2] if _H is None else _H
    wts = prep_weights(np.asarray(w_qkv, np.float32),
                       np.asarray(w_dw, np.float32),
                       np.asarray(w_proj, np.float32),
                       np.asarray(temperature, np.float32))
    nc = _get_nc(H)
    in_maps = []
    for i in range(b):
        m = {"x": np.ascontiguousarray(x[i]),
             "f": np.ascontiguousarray(f[i])}
        m.update(wts)
        in_maps.append(m)
    res = run_bass_kernel_spmd(nc, in_maps, core_ids=list(range(b)),
                               trace=_trace)
    out = np.stack([res.results[i]["out"] for i in range(b)], axis=0)
    kernel.last_results = res
    return out
